# revision 1
# baseline (speedup 1.0000x reference)
"""DefocusLKPN Trainium2 kernel.

Computes, per batch element (reference semantics):
    r      = clip(alpha * defocus + tanh(unet[:,100]), 0, 3)
    disk_k = sigmoid(5*(r - dist_k))            (25 taps, 6 distinct dists)
    w_ck   = exp(l_ck) * disk_k                 (l = unet[:, :100] logits)
    out_c  = sum_k w_ck * patch_ck / sum_k w_ck + x_c

Identity used: the global factor 2 of 2*sigmoid cancels in the num/den
ratio, so w_ck = exp(l_ck) * sigmoid(5*(r - dist_k)) works directly; sigmoid
(rather than 1 + tanh) keeps full relative precision for small disk weights
in fp16.
The softmax normalizer of the reference also cancels exactly (the EPS clamp
in the reference is never active for logits of this scale since the center
tap's disk mask is >= 0.5).

Sharding: pure data parallel, batch 16 -> 2 per core across 8 cores.

Per-core layout: partition dim = H (128); free dim packs (b, w) = 256 for
pixel planes and (k, b, w) for the 25-tap weight planes.  The 5x5 unfold is
realized as 5 row-shifted, column-padded copies of x in SBUF (vertical halo)
plus free-dim offsets (horizontal halo); the k-reduction runs on the tensor
engine as identity-matmul accumulation into PSUM (bf16 operands, f32
accumulation).  Only the tap weights and patches are bf16; the radius chain,
the normalization (num/den) and the final '+ x' stay f32.  fp16 (not bf16):
the 10-bit mantissa keeps the weighted-average error ~3e-4 of scale.
"""

import sys

sys.path.insert(0, "/opt/trn_rl_repo")

import numpy as np

import concourse.bass as bass
import concourse.mybir as mybir
from concourse.tile import TileContext
from concourse.bass_utils import run_bass_kernel_spmd

F32 = mybir.dt.float32
BF16 = mybir.dt.bfloat16
FP16 = mybir.dt.float16
AF = mybir.ActivationFunctionType
ALU = mybir.AluOpType

# dtype of the tap-weight / patch pipeline (lexp, s6, w, xs, m, identity).
MM_DT = FP16

N_CORES = 8
B, C, H, W = 16, 4, 128, 128
BL = B // N_CORES            # 2 batch elements per core
KK = 25
BW = BL * W                  # 256: (b, w) free block
WP = W + 4                   # 132: padded width per (b, c) block

# distinct tap distances; k = (dy+2)*5 + (dx+2)
DISTS = [0.0, 1.0, np.sqrt(2.0), 2.0, np.sqrt(5.0), np.sqrt(8.0)]
# (dist_index, base_k, [(step, count), ...]): {base + i*s1 + j*s2} lists the
# taps sharing that dist.
GROUPS = [
    (0, 12, []),                    # dist 0:      {12}
    (1, 7, [(6, 2), (4, 2)]),       # dist 1:      {7, 11, 13, 17}
    (2, 6, [(10, 2), (2, 2)]),      # dist sqrt2:  {6, 8, 16, 18}
    (3, 2, [(12, 2), (8, 2)]),      # dist 2:      {2, 10, 14, 22}
    (4, 5, [(10, 2), (4, 2)]),      # dist sqrt5:  {5, 9, 15, 19}
    (4, 1, [(20, 2), (2, 2)]),      # dist sqrt5:  {1, 3, 21, 23}
    (5, 0, [(20, 2), (4, 2)]),      # dist sqrt8:  {0, 4, 20, 24}
]


def _split_wide_waits(nc, max_waits=1):
    """The walrus build here accepts at most one semaphore wait per
    instruction; move extra waits onto preceding Drains on the same engine."""
    n = 0
    for func in nc.m.functions:
        for bb in func.blocks:
            out = []
            changed = False
            for ins in bb.instructions:
                si = ins.sync_info
                if si is not None and si.on_wait and len(si.on_wait) > max_waits:
                    waits = list(si.on_wait)
                    keep, rest = waits[:max_waits], waits[max_waits:]
                    for i in range(0, len(rest), max_waits):
                        n += 1
                        out.append(
                            mybir.InstDrain(
                                name=f"splitwait-{n}",
                                opcode="Drain",
                                engine=ins.engine,
                                sync_info=mybir.SyncInfo(
                                    on_wait=list(rest[i : i + max_waits]),
                                    on_update=[],
                                ),
                            )
                        )
                    si.on_wait = keep
                    changed = True
                out.append(ins)
            if changed:
                bb.instructions = out
    return n


def _ap(t, extra_off, dims):
    """AP over tile `t` keeping its partition dim, with free dims
    [[step, count], ...] in elements and an extra element offset."""
    return bass.AP(t.tensor, t.offset + extra_off, [list(t.ap[0])] + [list(d) for d in dims])


def _build():
    nc = bass.Bass("TRN2", num_devices=N_CORES)

    xl = nc.dram_tensor("x", [BL, C, H, W], F32, kind="ExternalInput")
    dfl = nc.dram_tensor("defocus", [BL, 1, H, W], F32, kind="ExternalInput")
    ul = nc.dram_tensor("unet", [BL, 4 * KK + 1, H, W], F32, kind="ExternalInput")
    al = nc.dram_tensor("alpha", [128, 1], F32, kind="ExternalInput")
    yl = nc.dram_tensor("y", [BL, C, H, W], F32, kind="ExternalOutput")

    ident_np = np.eye(128)
    ident_dram = nc.inline_tensor(
        ident_np.astype(mybir.dt.np(MM_DT)), name="ident"
    )

    UCH = ul.shape[1]          # 101
    HWr = H * W                # plane stride in DRAM
    # round-robin issuing engines for the big logit loads: each engine's
    # HWDGE has its own queue, so this parallelizes the HBM streams.
    dma_engines = [nc.sync, nc.scalar, nc.gpsimd]

    with TileContext(nc) as tc:
        with (
            tc.tile_pool(name="fix", bufs=1) as fix,
            tc.tile_pool(name="lp", bufs=2) as lp,
            tc.tile_pool(name="ep", bufs=2) as ep,
            tc.tile_pool(name="wp", bufs=2) as wpool,
            tc.tile_pool(name="mp", bufs=3) as mp,
            tc.tile_pool(name="op", bufs=2) as op,
            tc.tile_pool(name="ps", bufs=1, space="PSUM") as ps,
        ):
            # ---- constants / prologue ------------------------------------
            idt = fix.tile([128, 128], MM_DT)
            nc.sync.dma_start(out=idt[:], in_=ident_dram[:])
            asb = fix.tile([128, 1], F32)
            nc.sync.dma_start(out=asb[:], in_=al[:])

            df = fix.tile([128, BW], F32)
            nc.sync.dma_start(
                out=df[:],
                in_=bass.AP(dfl, 0, [[W, H], [HWr, BL], [1, W]]),
            )
            u100 = fix.tile([128, BW], F32)
            nc.sync.dma_start(
                out=u100[:],
                in_=bass.AP(ul, 100 * HWr, [[W, H], [UCH * HWr, BL], [1, W]]),
            )
            xf = fix.tile([128, BL * C * W], F32)
            nc.sync.dma_start(
                out=xf[:],
                in_=bass.AP(xl, 0, [[W, H], [HWr, BL * C], [1, W]]),
            )

            # radius = clip(alpha*defocus + tanh(u100), 0, 3)
            dtan = fix.tile([128, BW], F32)
            nc.scalar.activation(dtan[:], u100[:], AF.Tanh)
            r0 = fix.tile([128, BW], F32)
            nc.vector.scalar_tensor_tensor(r0[:], df[:], asb[:, :1], dtan[:], ALU.mult, ALU.add)
            rr = fix.tile([128, BW], F32)
            nc.vector.tensor_scalar(rr[:], r0[:], 0.0, 3.0, ALU.max, ALU.min)

            # s6[d] = sigmoid(5*r - 5*dist_d)   (6 planes, shared by all c)
            bt = fix.tile([128, 6], F32)
            for d in range(6):
                nc.gpsimd.memset(bt[:, d : d + 1], float(-5.0 * DISTS[d]))
            s6 = fix.tile([128, 6 * BW], MM_DT)
            for d in range(6):
                nc.scalar.activation(
                    s6[:, d * BW : (d + 1) * BW], rr[:], AF.Sigmoid,
                    bias=bt[:, d : d + 1], scale=5.0,
                )

            # x cast to the matmul dtype, then 5 row-shifted padded copies
            if MM_DT is F32:
                xb = xf
            else:
                xb = fix.tile([128, BL * C * W], MM_DT)
                nc.vector.tensor_copy(xb[:], xf[:])
            # xs: pad offset 2 (even dx slices start 4B-aligned);
            # xso: pad offset 3 (odd dx slices start 4B-aligned)
            xs, xso = [], []
            for pad, lst, nm in ((2, xs, "xs"), (3, xso, "xso")):
                for dyi, dy in enumerate((-2, -1, 0, 1, 2)):
                    xst = fix.tile([128, BL * C * WP], MM_DT, name=f"{nm}{dyi}")
                    nc.gpsimd.memset(xst[:], 0.0)
                    lo, hi = max(0, -dy), 128 - max(0, dy)
                    bsrc = _ap(xb, 0, [[W, BL * C], [1, W]])
                    bsrc = bass.AP(bsrc.tensor, bsrc.offset, bsrc.ap)
                    srcv = xb.rearrange("p (bc w) -> p bc w", bc=BL * C, w=W)[
                        lo + dy : hi + dy
                    ]
                    dstv = xst.rearrange("p (bc wp) -> p bc wp", bc=BL * C, wp=WP)[
                        lo:hi, :, pad : pad + W
                    ]
                    nc.sync.dma_start(out=dstv, in_=srcv)
                    lst.append(xst)

            # ---- per-channel main loop -----------------------------------
            # numden[c] accumulates [num | den] side by side; each matmul's
            # rhs is one tap's [m_k (256) | w_k (256)] block (N=512 = one
            # PSUM bank).  The [m|w] blocks are packed in per-(c,dy) tiles
            # (5 taps each) so dependencies stay fine-grained and the tensor
            # engine starts as soon as one dy-group is ready.
            KB = 2 * BW
            D2I = {0: 0, 1: 1, 2: 2, 4: 3, 5: 4, 8: 5}
            numdens = []
            for c in range(C):
                nd = ps.tile([128, 2 * BW], F32, name=f"numden{c}")
                numdens.append(nd)

                l = lp.tile([128, KK * BW], F32, name="l")
                for b in range(BL):
                    # three concurrent HW queues, ~1/3 of the planes each
                    for (k0, nk), dma_eng in zip(
                        ((0, 9), (9, 8), (17, 8)), dma_engines
                    ):
                        dma_eng.dma_start(
                            out=_ap(l, b * W + k0 * BW, [[BW, nk], [1, W]]),
                            in_=bass.AP(
                                ul, (c * KK + k0 + b * UCH) * HWr,
                                [[W, H], [HWr, nk], [1, W]],
                            ),
                        )
                lexp = ep.tile([128, KK * BW], MM_DT, name="lexp")
                nc.scalar.activation(lexp[:], l[:], AF.Exp)

                for dy in range(5):
                    d2 = (dy - 2) * (dy - 2)
                    mdy = mp.tile([128, 5 * KB], MM_DT, name="mdy")
                    # w_j = s6[d] * lexp, into the w half of each block;
                    # taps are symmetric in j: pairs {0,4}, {1,3}, single {2}
                    for j0, step, cnt, dd in (
                        (0, 4, 2, d2 + 4),
                        (1, 2, 2, d2 + 1),
                        (2, 1, 1, d2),
                    ):
                        pair = [[step * KB, cnt]] if cnt > 1 else []
                        lpair = [[step * BW, cnt]] if cnt > 1 else []
                        bdims = [[0, cnt]] if cnt > 1 else []
                        nc.vector.tensor_tensor(
                            _ap(mdy, j0 * KB + BW, pair + [[1, BW]]),
                            _ap(s6, D2I[dd] * BW, bdims + [[1, BW]]),
                            _ap(lexp, (dy * 5 + j0) * BW, lpair + [[1, BW]]),
                            ALU.mult,
                        )
                    # m_j = w_j * xs; even/odd dx split keeps fp16 slice
                    # starts 4B-aligned for the DVE 2x mode
                    for b in range(BL):
                        nc.vector.tensor_tensor(
                            _ap(mdy, b * W, [[2 * KB, 3], [1, W]]),
                            _ap(mdy, BW + b * W, [[2 * KB, 3], [1, W]]),
                            _ap(xs[dy], c * WP + b * C * WP, [[2, 3], [1, W]]),
                            ALU.mult,
                        )
                        nc.vector.tensor_tensor(
                            _ap(mdy, KB + b * W, [[2 * KB, 2], [1, W]]),
                            _ap(mdy, KB + BW + b * W, [[2 * KB, 2], [1, W]]),
                            _ap(xso[dy], c * WP + b * C * WP + 2, [[2, 2], [1, W]]),
                            ALU.mult,
                        )
                    for j in range(5):
                        nc.tensor.matmul(
                            nd[:], idt[:], mdy[:, j * KB : (j + 1) * KB],
                            start=(dy == 0 and j == 0), stop=(dy == 4 and j == 4),
                        )

            # ---- epilogue: out_c = num/den + x ---------------------------
            for c in range(C):
                rden = op.tile([128, BW], F32, name="rden")
                nc.vector.reciprocal(rden[:], numdens[c][:, BW : 2 * BW])
                o1 = op.tile([128, BW], F32, name="o1")
                nc.vector.scalar_tensor_tensor(
                    o1[:], numdens[c][:, 0:BW], 1.0, rden[:], ALU.bypass, ALU.mult
                )
                o2 = op.tile([128, BW], F32, name="o2")
                nc.vector.tensor_tensor(
                    o2[:], o1[:], _ap(xf, c * W, [[C * W, BL], [1, W]]), ALU.add
                )
                nc.scalar.dma_start(
                    out=bass.AP(yl, c * HWr, [[W, H], [C * HWr, BL], [1, W]]),
                    in_=o2[:],
                )

    _split_wide_waits(nc)
    return nc


_NC_CACHE = None


def _get_nc():
    global _NC_CACHE
    if _NC_CACHE is None:
        _NC_CACHE = _build()
    return _NC_CACHE


def _make_in_maps(x, defocus_map, unet_out, alpha):
    x = np.ascontiguousarray(x, dtype=np.float32)
    defocus_map = np.ascontiguousarray(defocus_map, dtype=np.float32)
    unet_out = np.ascontiguousarray(unet_out, dtype=np.float32)
    alpha_b = np.full((128, 1), np.float32(np.asarray(alpha).reshape(-1)[0]))
    in_maps = []
    for core in range(N_CORES):
        s = slice(core * BL, (core + 1) * BL)
        in_maps.append(
            {
                "x": x[s],
                "defocus": defocus_map[s],
                "unet": unet_out[s],
                "alpha": alpha_b,
            }
        )
    return in_maps


def run(x, defocus_map, unet_out, alpha, **spmd_kwargs):
    """Run the kernel; returns (output, BassKernelResults)."""
    nc = _get_nc()
    in_maps = _make_in_maps(x, defocus_map, unet_out, alpha)
    res = run_bass_kernel_spmd(nc, in_maps, list(range(N_CORES)), **spmd_kwargs)
    out = np.concatenate([res.results[i]["y"] for i in range(N_CORES)], axis=0)
    return out.astype(np.float32), res


def kernel(x, defocus_map, unet_out, alpha):
    return run(x, defocus_map, unet_out, alpha)[0]



# revision 3
# speedup vs baseline: 1.5339x; 1.5339x over previous
"""DefocusLKPN Trainium2 kernel.

Computes, per batch element (reference semantics):
    r      = clip(alpha * defocus + tanh(unet[:,100]), 0, 3)
    disk_k = sigmoid(5*(r - dist_k))            (25 taps, 6 distinct dists)
    w_ck   = exp(l_ck) * disk_k                 (l = unet[:, :100] logits)
    out_c  = sum_k w_ck * patch_ck / sum_k w_ck + x_c

The softmax normalizer of the reference cancels exactly in num/den (the EPS
clamp is never active since the center tap's disk mask is >= 0.5).

Sharding: pure data parallel, batch 16 -> 2 per core across 8 cores.

Per-core layout: partition dim = H (128); free dim packs (b, w) = 256 for
pixel planes and (k, b, w) for the 25-tap weight planes.  The 5x5 unfold is
realized as 5 row-shifted, column-padded copies of x in SBUF (vertical halo)
plus free-dim offsets (horizontal halo).  The row shift itself runs on the
tensor engine (shift-matrix matmuls into PSUM + PSUM->SBUF casts): SBUF->SBUF
shift DMAs measured 14-31 us each on HW and stalled the sync HWDGE FIFO, so
no DMA touches the halo build.  The k-reduction runs on the tensor engine as
identity-matmul accumulation into PSUM (fp16 operands, f32 accumulation).
The disk masks are broadcast once into a 25-tap plane (s25) so the tap-weight
multiplies collapse into a handful of large strided DVE ops; fp16 (not bf16):
the 10-bit mantissa keeps the weighted-average error ~3e-4 of scale.
Logit HBM loads round-robin the three DMA-capable queues (sync/scalar HWDGE,
gpsimd SWDGE) so the 12.5 MiB/core stream saturates HBM.
"""

import sys

sys.path.insert(0, "/opt/trn_rl_repo")

import numpy as np

import concourse.bass as bass
import concourse.mybir as mybir
from concourse.tile import TileContext
from concourse.bass_utils import run_bass_kernel_spmd

F32 = mybir.dt.float32
FP16 = mybir.dt.float16
AF = mybir.ActivationFunctionType
ALU = mybir.AluOpType

MM_DT = FP16

N_CORES = 8
B, C, H, W = 16, 4, 128, 128
BL = B // N_CORES            # 2 batch elements per core
KK = 25
BW = BL * W                  # 256: (b, w) free block
KB = 2 * BW                  # 512: [m | w] block per tap
WP = W + 4                   # 132: padded width per (b, c) block
SDY = BL * C * WP            # 1056: one dy-plane of padded x
DYS = (-2, -1, 0, 1, 2)
KGROUPS = ((0, 9), (9, 8), (17, 8))   # logit planes per DMA queue

# distinct tap distances; k = (dy+2)*5 + (dx+2)
DISTS = [0.0, 1.0, np.sqrt(2.0), 2.0, np.sqrt(5.0), np.sqrt(8.0)]
# (dist_index, base_k, [(step, count), ...]): {base + i*s1 + j*s2} lists the
# taps sharing that dist.
GROUPS = [
    (0, 12, []),                    # dist 0:      {12}
    (1, 7, [(6, 2), (4, 2)]),       # dist 1:      {7, 11, 13, 17}
    (2, 6, [(10, 2), (2, 2)]),      # dist sqrt2:  {6, 8, 16, 18}
    (3, 2, [(12, 2), (8, 2)]),      # dist 2:      {2, 10, 14, 22}
    (4, 5, [(10, 2), (4, 2)]),      # dist sqrt5:  {5, 9, 15, 19}
    (4, 1, [(20, 2), (2, 2)]),      # dist sqrt5:  {1, 3, 21, 23}
    (5, 0, [(20, 2), (4, 2)]),      # dist sqrt8:  {0, 4, 20, 24}
]


def _split_wide_waits(nc, max_waits=1):
    """The walrus build here accepts at most one semaphore wait per
    instruction; move extra waits onto preceding Drains on the same engine."""
    n = 0
    for func in nc.m.functions:
        for bb in func.blocks:
            out = []
            changed = False
            for ins in bb.instructions:
                si = ins.sync_info
                if si is not None and si.on_wait and len(si.on_wait) > max_waits:
                    waits = list(si.on_wait)
                    keep, rest = waits[:max_waits], waits[max_waits:]
                    for i in range(0, len(rest), max_waits):
                        n += 1
                        out.append(
                            mybir.InstDrain(
                                name=f"splitwait-{n}",
                                opcode="Drain",
                                engine=ins.engine,
                                sync_info=mybir.SyncInfo(
                                    on_wait=list(rest[i : i + max_waits]),
                                    on_update=[],
                                ),
                            )
                        )
                    si.on_wait = keep
                    changed = True
                out.append(ins)
            if changed:
                bb.instructions = out
    return n


def _ap(t, extra_off, dims):
    """AP over tile `t` keeping its partition dim, with free dims
    [[step, count], ...] in elements and an extra element offset."""
    return bass.AP(t.tensor, t.offset + extra_off, [list(t.ap[0])] + [list(d) for d in dims])


def _shift_mats():
    """[128, 5*128] fp16: column block dyi holds S with S[q, p] = 1 iff
    q == p + dy, so out[p] = (S.T @ x)[p] = x[p + dy] (zero out of range)."""
    sh = np.zeros((128, 5 * 128), dtype=np.float16)
    for dyi, dy in enumerate(DYS):
        for p in range(128):
            q = p + dy
            if 0 <= q < 128:
                sh[q, dyi * 128 + p] = 1.0
    return sh


def _build():
    nc = bass.Bass("TRN2", num_devices=N_CORES)

    xl = nc.dram_tensor("x", [BL, C, H, W], F32, kind="ExternalInput")
    dfl = nc.dram_tensor("defocus", [BL, 1, H, W], F32, kind="ExternalInput")
    ul = nc.dram_tensor("unet", [BL, 4 * KK + 1, H, W], F32, kind="ExternalInput")
    al = nc.dram_tensor("alpha", [128, 1], F32, kind="ExternalInput")
    yl = nc.dram_tensor("y", [BL, C, H, W], F32, kind="ExternalOutput")

    shifts_dram = nc.inline_tensor(_shift_mats(), name="shifts")

    UCH = ul.shape[1]          # 101
    HWr = H * W                # plane stride in DRAM
    dma_engines = [None, None, None]  # filled inside (sync/scalar/gpsimd)

    with TileContext(nc) as tc:
        with (
            tc.tile_pool(name="fix", bufs=1) as fix,
            tc.tile_pool(name="lp", bufs=2) as lp,
            tc.tile_pool(name="ep", bufs=2) as ep,
            tc.tile_pool(name="mp", bufs=2) as mp,
            tc.tile_pool(name="op", bufs=2) as op,
            tc.tile_pool(name="ps", bufs=1, space="PSUM") as ps,
            tc.tile_pool(name="psx", bufs=2, space="PSUM") as psx,
        ):
            dma_engines = [nc.sync, nc.scalar, nc.gpsimd]
            # ---- constants / prologue ------------------------------------
            shf = fix.tile([128, 5 * 128], MM_DT)
            nc.sync.dma_start(out=shf[:], in_=shifts_dram[:])
            idt = shf[:, 2 * 128 : 3 * 128]          # dy=0 block == identity
            asb = fix.tile([128, 1], F32)
            nc.sync.dma_start(out=asb[:], in_=al[:])

            df = fix.tile([128, BW], F32)
            nc.sync.dma_start(
                out=df[:],
                in_=bass.AP(dfl, 0, [[W, H], [HWr, BL], [1, W]]),
            )
            u100 = fix.tile([128, BW], F32)
            nc.sync.dma_start(
                out=u100[:],
                in_=bass.AP(ul, 100 * HWr, [[W, H], [UCH * HWr, BL], [1, W]]),
            )
            xf = fix.tile([128, BL * C * W], F32)
            nc.sync.dma_start(
                out=xf[:],
                in_=bass.AP(xl, 0, [[W, H], [HWr, BL * C], [1, W]]),
            )

            # radius = clip(alpha*defocus + tanh(u100), 0, 3)
            dtan = fix.tile([128, BW], F32)
            nc.scalar.activation(dtan[:], u100[:], AF.Tanh)
            r0 = fix.tile([128, BW], F32)
            nc.vector.scalar_tensor_tensor(r0[:], df[:], asb[:, :1], dtan[:], ALU.mult, ALU.add)
            rr = fix.tile([128, BW], F32)
            nc.vector.tensor_scalar(rr[:], r0[:], 0.0, 3.0, ALU.max, ALU.min)

            # s6[d] = sigmoid(5*r - 5*dist_d)   (6 planes, shared by all c)
            bt = fix.tile([128, 6], F32)
            for d in range(6):
                nc.gpsimd.memset(bt[:, d : d + 1], float(-5.0 * DISTS[d]))
            s6 = fix.tile([128, 6 * BW], MM_DT)
            for d in range(6):
                nc.scalar.activation(
                    s6[:, d * BW : (d + 1) * BW], rr[:], AF.Sigmoid,
                    bias=bt[:, d : d + 1], scale=5.0,
                )
            # broadcast the 6 dist planes into per-tap order (idle gpsimd)
            s25 = fix.tile([128, KK * BW], MM_DT)
            for d, base, steps in GROUPS:
                sdims = [[s * BW, cnt] for s, cnt in steps] + [[1, BW]]
                bdims = [[0, cnt] for _, cnt in steps] + [[1, BW]]
                nc.gpsimd.tensor_copy(
                    _ap(s25, base * BW, sdims), _ap(s6, d * BW, bdims)
                )

            # x cast to fp16, then 5 row-shifted padded copies via the PE:
            # xps[p] = x[p+dy] (shift-matrix matmul), cast into the padded
            # tiles.  xs_all: pad 2 (even dx slices 4B-aligned); xso_all:
            # pad 3 (odd dx slices 4B-aligned).
            xb = fix.tile([128, BL * C * W], MM_DT)
            nc.vector.tensor_copy(xb[:], xf[:])
            xs_all = fix.tile([128, 5 * SDY], MM_DT)
            xso_all = fix.tile([128, 5 * SDY], MM_DT)
            for t, pad in ((xs_all, 2), (xso_all, 3)):
                nc.gpsimd.memset(_ap(t, 0, [[SDY, 5], [WP, BL * C], [1, pad]]), 0.0)
                nc.gpsimd.memset(
                    _ap(t, pad + W, [[SDY, 5], [WP, BL * C], [1, WP - pad - W]]), 0.0
                )
            for dyi in range(5):
                xps = psx.tile([128, BL * C * W], F32, name="xps")
                for h in range(2):
                    nc.tensor.matmul(
                        xps[:, h * 512 : (h + 1) * 512],
                        shf[:, dyi * 128 : (dyi + 1) * 128],
                        xb[:, h * 512 : (h + 1) * 512],
                        start=True, stop=True,
                    )
                for t, pad in ((xs_all, 2), (xso_all, 3)):
                    nc.vector.tensor_copy(
                        _ap(t, dyi * SDY + pad, [[WP, BL * C], [1, W]]),
                        _ap(xps, 0, [[W, BL * C], [1, W]]),
                    )

            # ---- per-channel main loop -----------------------------------
            # numden[c] accumulates [num | den]; each matmul's rhs is one
            # tap's [m_k (256) | w_k (256)] block (N=512 = one PSUM bank).
            for c in range(C):
                nd = ps.tile([128, KB], F32, name=f"numden{c}")

                l = lp.tile([128, KK * BW], F32, name="l")
                for b in range(BL):
                    # three concurrent HW queues, ~1/3 of the planes each
                    for (k0, nk), dma_eng in zip(KGROUPS, dma_engines):
                        dma_eng.dma_start(
                            out=_ap(l, b * W + k0 * BW, [[BW, nk], [1, W]]),
                            in_=bass.AP(
                                ul, (c * KK + k0 + b * UCH) * HWr,
                                [[W, H], [HWr, nk], [1, W]],
                            ),
                        )
                lexp = ep.tile([128, KK * BW], MM_DT, name="lexp")
                for k0, nk in KGROUPS:
                    nc.scalar.activation(
                        lexp[:, k0 * BW : (k0 + nk) * BW],
                        l[:, k0 * BW : (k0 + nk) * BW], AF.Exp,
                    )

                m = mp.tile([128, KK * KB], MM_DT, name="mall")
                # w_k = s25_k * lexp_k into the w half of each tap block
                for k0, nk in KGROUPS:
                    nc.vector.tensor_tensor(
                        _ap(m, k0 * KB + BW, [[KB, nk], [1, BW]]),
                        _ap(s25, k0 * BW, [[BW, nk], [1, BW]]),
                        _ap(lexp, k0 * BW, [[BW, nk], [1, BW]]),
                        ALU.mult,
                    )
                # m_k = w_k * x[h+dy, w+dx]; all 25 taps in 2 ops per b
                # (even dx from xs_all, odd dx from xso_all: 4B-aligned)
                for b in range(BL):
                    nc.vector.tensor_tensor(
                        _ap(m, b * W, [[5 * KB, 5], [2 * KB, 3], [1, W]]),
                        _ap(m, BW + b * W, [[5 * KB, 5], [2 * KB, 3], [1, W]]),
                        _ap(xs_all, (b * C + c) * WP, [[SDY, 5], [2, 3], [1, W]]),
                        ALU.mult,
                    )
                    nc.vector.tensor_tensor(
                        _ap(m, KB + b * W, [[5 * KB, 5], [2 * KB, 2], [1, W]]),
                        _ap(m, KB + BW + b * W, [[5 * KB, 5], [2 * KB, 2], [1, W]]),
                        _ap(xso_all, (b * C + c) * WP + 2, [[SDY, 5], [2, 2], [1, W]]),
                        ALU.mult,
                    )
                for k in range(KK):
                    nc.tensor.matmul(
                        nd[:], idt, m[:, k * KB : (k + 1) * KB],
                        start=(k == 0), stop=(k == KK - 1),
                    )

                # epilogue: out_c = num/den + x (interleaved per channel so
                # the tail drains while later channels stream)
                rden = op.tile([128, BW], F32, name="rden")
                nc.vector.reciprocal(rden[:], nd[:, BW : 2 * BW])
                o1 = op.tile([128, BW], F32, name="o1")
                nc.vector.scalar_tensor_tensor(
                    o1[:], nd[:, 0:BW], 1.0, rden[:], ALU.bypass, ALU.mult
                )
                o2 = op.tile([128, BW], F32, name="o2")
                nc.vector.tensor_tensor(
                    o2[:], o1[:], _ap(xf, c * W, [[C * W, BL], [1, W]]), ALU.add
                )
                nc.gpsimd.dma_start(
                    out=bass.AP(yl, c * HWr, [[W, H], [C * HWr, BL], [1, W]]),
                    in_=o2[:],
                )

    _split_wide_waits(nc)
    return nc


_NC_CACHE = None


def _get_nc():
    global _NC_CACHE
    if _NC_CACHE is None:
        _NC_CACHE = _build()
    return _NC_CACHE


def _make_in_maps(x, defocus_map, unet_out, alpha):
    x = np.ascontiguousarray(x, dtype=np.float32)
    defocus_map = np.ascontiguousarray(defocus_map, dtype=np.float32)
    unet_out = np.ascontiguousarray(unet_out, dtype=np.float32)
    alpha_b = np.full((128, 1), np.float32(np.asarray(alpha).reshape(-1)[0]))
    in_maps = []
    for core in range(N_CORES):
        s = slice(core * BL, (core + 1) * BL)
        in_maps.append(
            {
                "x": x[s],
                "defocus": defocus_map[s],
                "unet": unet_out[s],
                "alpha": alpha_b,
            }
        )
    return in_maps


def run(x, defocus_map, unet_out, alpha, **spmd_kwargs):
    """Run the kernel; returns (output, BassKernelResults)."""
    nc = _get_nc()
    in_maps = _make_in_maps(x, defocus_map, unet_out, alpha)
    res = run_bass_kernel_spmd(nc, in_maps, list(range(N_CORES)), **spmd_kwargs)
    out = np.concatenate([res.results[i]["y"] for i in range(N_CORES)], axis=0)
    return out.astype(np.float32), res


def kernel(x, defocus_map, unet_out, alpha):
    return run(x, defocus_map, unet_out, alpha)[0]


# revision 10
# speedup vs baseline: 1.7490x; 1.1403x over previous
"""DefocusLKPN Trainium2 kernel.

Computes, per batch element (reference semantics):
    r      = clip(alpha * defocus + tanh(unet[:,100]), 0, 3)
    disk_k = sigmoid(5*(r - dist_k))            (25 taps, 6 distinct dists)
    w_ck   = exp(l_ck) * disk_k                 (l = unet[:, :100] logits)
    out_c  = sum_k w_ck * patch_ck / sum_k w_ck + x_c

The softmax normalizer of the reference cancels exactly in num/den (the EPS
clamp is never active since the center tap's disk mask is >= 0.5).

Sharding: pure data parallel, batch 16 -> 2 per core across 8 cores.

Per-core layout: partition dim = H (128); free dim packs (b, w) = 256 for
pixel planes and (k, b, w) for the 25-tap weight planes.  The 5x5 unfold is
realized as 5 row-shifted, column-padded copies of x in SBUF (vertical halo)
plus free-dim offsets (horizontal halo).  The row shift itself runs on the
tensor engine (shift-matrix matmuls into PSUM + PSUM->SBUF casts): SBUF->SBUF
shift DMAs measured 14-31 us each on HW and stalled the sync HWDGE FIFO, so
no DMA touches the halo build.  The k-reduction runs on the tensor engine as
identity-matmul accumulation into PSUM (fp16 operands, f32 accumulation).
The disk masks are broadcast once into a 25-tap plane (s25) so the tap-weight
multiplies collapse into a handful of large strided DVE ops; fp16 (not bf16):
the 10-bit mantissa keeps the weighted-average error ~3e-4 of scale.
Logit HBM loads round-robin the three DMA-capable queues (sync/scalar HWDGE,
gpsimd SWDGE) so the 12.5 MiB/core stream saturates HBM.
"""

import sys

sys.path.insert(0, "/opt/trn_rl_repo")

import numpy as np

import concourse.bass as bass
import concourse.mybir as mybir
from concourse.tile import TileContext
from concourse.bass_utils import run_bass_kernel_spmd

F32 = mybir.dt.float32
FP16 = mybir.dt.float16
AF = mybir.ActivationFunctionType
ALU = mybir.AluOpType

MM_DT = FP16

N_CORES = 8
B, C, H, W = 16, 4, 128, 128
BL = B // N_CORES            # 2 batch elements per core
KK = 25
BW = BL * W                  # 256: (b, w) free block
KB = 2 * BW                  # 512: [m | w] block per tap
WP = W + 4                   # 132: padded width per (b, c) block
SDY = BL * C * WP            # 1056: one dy-plane of padded x
DYS = (-2, -1, 0, 1, 2)
KGROUPS = ((0, 9), (9, 8), (17, 8))   # logit planes per DMA queue

# distinct tap distances; k = (dy+2)*5 + (dx+2)
DISTS = [0.0, 1.0, np.sqrt(2.0), 2.0, np.sqrt(5.0), np.sqrt(8.0)]
# (dist_index, base_k, [(step, count), ...]): {base + i*s1 + j*s2} lists the
# taps sharing that dist.
GROUPS = [
    (0, 12, []),                    # dist 0:      {12}
    (1, 7, [(6, 2), (4, 2)]),       # dist 1:      {7, 11, 13, 17}
    (2, 6, [(10, 2), (2, 2)]),      # dist sqrt2:  {6, 8, 16, 18}
    (3, 2, [(12, 2), (8, 2)]),      # dist 2:      {2, 10, 14, 22}
    (4, 5, [(10, 2), (4, 2)]),      # dist sqrt5:  {5, 9, 15, 19}
    (4, 1, [(20, 2), (2, 2)]),      # dist sqrt5:  {1, 3, 21, 23}
    (5, 0, [(20, 2), (4, 2)]),      # dist sqrt8:  {0, 4, 20, 24}
]


def _split_wide_waits(nc, max_waits=1):
    """The walrus build here accepts at most one semaphore wait per
    instruction; move extra waits onto preceding Drains on the same engine."""
    n = 0
    for func in nc.m.functions:
        for bb in func.blocks:
            out = []
            changed = False
            for ins in bb.instructions:
                si = ins.sync_info
                if si is not None and si.on_wait and len(si.on_wait) > max_waits:
                    waits = list(si.on_wait)
                    keep, rest = waits[:max_waits], waits[max_waits:]
                    for i in range(0, len(rest), max_waits):
                        n += 1
                        out.append(
                            mybir.InstDrain(
                                name=f"splitwait-{n}",
                                opcode="Drain",
                                engine=ins.engine,
                                sync_info=mybir.SyncInfo(
                                    on_wait=list(rest[i : i + max_waits]),
                                    on_update=[],
                                ),
                            )
                        )
                    si.on_wait = keep
                    changed = True
                out.append(ins)
            if changed:
                bb.instructions = out
    return n


def _ap(t, extra_off, dims):
    """AP over tile `t` keeping its partition dim, with free dims
    [[step, count], ...] in elements and an extra element offset."""
    return bass.AP(t.tensor, t.offset + extra_off, [list(t.ap[0])] + [list(d) for d in dims])


def _shift_mats():
    """[128, 5*128] fp16: column block dyi holds S with S[q, p] = 1 iff
    q == p + dy, so out[p] = (S.T @ x)[p] = x[p + dy] (zero out of range)."""
    sh = np.zeros((128, 5 * 128), dtype=np.float16)
    for dyi, dy in enumerate(DYS):
        for p in range(128):
            q = p + dy
            if 0 <= q < 128:
                sh[q, dyi * 128 + p] = 1.0
    return sh


def _build():
    nc = bass.Bass("TRN2", num_devices=N_CORES)

    xl = nc.dram_tensor("x", [BL, C, H, W], F32, kind="ExternalInput")
    dfl = nc.dram_tensor("defocus", [BL, 1, H, W], F32, kind="ExternalInput")
    ul = nc.dram_tensor("unet", [BL, 4 * KK + 1, H, W], F32, kind="ExternalInput")
    al = nc.dram_tensor("alpha", [128, 1], F32, kind="ExternalInput")
    yl = nc.dram_tensor("y", [BL, C, H, W], F32, kind="ExternalOutput")

    shifts_dram = nc.inline_tensor(_shift_mats(), name="shifts")

    UCH = ul.shape[1]          # 101
    HWr = H * W                # plane stride in DRAM
    dma_engines = [None, None, None]  # filled inside (sync/scalar/gpsimd)

    with TileContext(nc) as tc:
        with (
            tc.tile_pool(name="fix", bufs=1) as fix,
            tc.tile_pool(name="lp", bufs=2) as lp,
            tc.tile_pool(name="ep", bufs=2) as ep,
            tc.tile_pool(name="mp", bufs=2) as mp,
            tc.tile_pool(name="op", bufs=2) as op,
            tc.tile_pool(name="ps", bufs=1, space="PSUM") as ps,
            tc.tile_pool(name="psx", bufs=2, space="PSUM") as psx,
        ):
            dma_engines = [nc.sync, nc.scalar, nc.gpsimd]
            # ---- constants / prologue ------------------------------------
            # radius-chain inputs first so their bytes hit the wire earliest
            asb = fix.tile([128, 1], F32)
            nc.sync.dma_start(out=asb[:], in_=al[:])
            df = fix.tile([128, BW], F32)
            nc.sync.dma_start(
                out=df[:],
                in_=bass.AP(dfl, 0, [[W, H], [HWr, BL], [1, W]]),
            )
            u100 = fix.tile([128, BW], F32)
            nc.sync.dma_start(
                out=u100[:],
                in_=bass.AP(ul, 100 * HWr, [[W, H], [UCH * HWr, BL], [1, W]]),
            )
            xf = fix.tile([128, BL * C * W], F32)
            nc.sync.dma_start(
                out=xf[:],
                in_=bass.AP(xl, 0, [[W, H], [HWr, BL * C], [1, W]]),
            )
            shf = fix.tile([128, 5 * 128], MM_DT)
            nc.sync.dma_start(out=shf[:], in_=shifts_dram[:])
            idt = shf[:, 2 * 128 : 3 * 128]          # dy=0 block == identity

            # x cast first so it leads the DVE queue (depends only on xf)
            xb = fix.tile([128, BL * C * W], MM_DT)
            nc.vector.tensor_copy(xb[:], xf[:])

            # radius = clip(alpha*defocus + tanh(u100), 0, 3)
            dtan = fix.tile([128, BW], F32)
            nc.scalar.activation(dtan[:], u100[:], AF.Tanh)
            r0 = fix.tile([128, BW], F32)
            nc.vector.scalar_tensor_tensor(r0[:], df[:], asb[:, :1], dtan[:], ALU.mult, ALU.add)
            rr = fix.tile([128, BW], F32)
            nc.vector.tensor_scalar(rr[:], r0[:], 0.0, 3.0, ALU.max, ALU.min)

            # s6[d] = sigmoid(5*r - 5*dist_d)   (6 planes, shared by all c)
            bt = fix.tile([128, 6], F32)
            for d in range(6):
                nc.gpsimd.memset(bt[:, d : d + 1], float(-5.0 * DISTS[d]))
            s6 = fix.tile([128, 6 * BW], MM_DT)
            for d in range(6):
                nc.scalar.activation(
                    s6[:, d * BW : (d + 1) * BW], rr[:], AF.Sigmoid,
                    bias=bt[:, d : d + 1], scale=5.0,
                )
            # broadcast the 6 dist planes into per-tap order (cheap on DVE;
            # gpsimd's Q7 runs ~4 ns/elem and serialized 21 us here)
            s25 = fix.tile([128, KK * BW], MM_DT)
            for d, base, steps in GROUPS:
                sdims = [[s * BW, cnt] for s, cnt in steps] + [[1, BW]]
                bdims = [[0, cnt] for _, cnt in steps] + [[1, BW]]
                nc.vector.tensor_copy(
                    _ap(s25, base * BW, sdims), _ap(s6, d * BW, bdims)
                )

            # 5 row-shifted padded copies of x: dy=0 is a plain strided copy
            # of xb; dy!=0 run on the PE (shift-matrix matmul into PSUM, then
            # cast into the padded tile).  Even dx taps read 4B-aligned
            # slices (DVE 2x); odd dx taps read the same tile at odd offsets
            # in 1x mode (cheaper than keeping a second odd-aligned copy).
            xs_all = fix.tile([128, 5 * SDY], MM_DT)
            nc.gpsimd.memset(_ap(xs_all, 0, [[SDY, 5], [WP, BL * C], [1, 2]]), 0.0)
            nc.gpsimd.memset(
                _ap(xs_all, 2 + W, [[SDY, 5], [WP, BL * C], [1, 2]]), 0.0
            )
            nc.vector.tensor_copy(
                _ap(xs_all, 2 * SDY + 2, [[WP, BL * C], [1, W]]),
                _ap(xb, 0, [[W, BL * C], [1, W]]),
            )
            for dyi in (0, 1, 3, 4):
                xps = psx.tile([128, BL * C * W], F32, name="xps")
                for h in range(2):
                    nc.tensor.matmul(
                        xps[:, h * 512 : (h + 1) * 512],
                        shf[:, dyi * 128 : (dyi + 1) * 128],
                        xb[:, h * 512 : (h + 1) * 512],
                        start=True, stop=True,
                    )
                nc.vector.tensor_copy(
                    _ap(xs_all, dyi * SDY + 2, [[WP, BL * C], [1, W]]),
                    _ap(xps, 0, [[W, BL * C], [1, W]]),
                )

            # ---- per-channel main loop -----------------------------------
            # numden[c] accumulates [num | den]; each matmul's rhs is one
            # tap's [m_k (256) | w_k (256)] block (N=512 = one PSUM bank).
            for c in range(C):
                nd = ps.tile([128, KB], F32, name=f"numden{c}")

                l = lp.tile([128, KK * BW], F32, name="l")
                for b in range(BL):
                    # three concurrent HW queues, ~1/3 of the planes each
                    for (k0, nk), dma_eng in zip(KGROUPS, dma_engines):
                        dma_eng.dma_start(
                            out=_ap(l, b * W + k0 * BW, [[BW, nk], [1, W]]),
                            in_=bass.AP(
                                ul, (c * KK + k0 + b * UCH) * HWr,
                                [[W, H], [HWr, nk], [1, W]],
                            ),
                        )
                lexp = ep.tile([128, KK * BW], MM_DT, name="lexp")
                for k0, nk in KGROUPS:
                    nc.scalar.activation(
                        lexp[:, k0 * BW : (k0 + nk) * BW],
                        l[:, k0 * BW : (k0 + nk) * BW], AF.Exp,
                    )

                m = mp.tile([128, KK * KB], MM_DT, name="mall")
                # w_k = s25_k * lexp_k into the w half of each tap block
                for k0, nk in KGROUPS:
                    nc.vector.tensor_tensor(
                        _ap(m, k0 * KB + BW, [[KB, nk], [1, BW]]),
                        _ap(s25, k0 * BW, [[BW, nk], [1, BW]]),
                        _ap(lexp, k0 * BW, [[BW, nk], [1, BW]]),
                        ALU.mult,
                    )
                # m_k = w_k * x[h+dy, w+dx]; all 25 taps in 2 ops per b
                # (even dx 4B-aligned = DVE 2x; odd dx offsets run 1x)
                for b in range(BL):
                    nc.vector.tensor_tensor(
                        _ap(m, b * W, [[5 * KB, 5], [2 * KB, 3], [1, W]]),
                        _ap(m, BW + b * W, [[5 * KB, 5], [2 * KB, 3], [1, W]]),
                        _ap(xs_all, (b * C + c) * WP, [[SDY, 5], [2, 3], [1, W]]),
                        ALU.mult,
                    )
                    nc.vector.tensor_tensor(
                        _ap(m, KB + b * W, [[5 * KB, 5], [2 * KB, 2], [1, W]]),
                        _ap(m, KB + BW + b * W, [[5 * KB, 5], [2 * KB, 2], [1, W]]),
                        _ap(xs_all, (b * C + c) * WP + 1, [[SDY, 5], [2, 2], [1, W]]),
                        ALU.mult,
                    )
                for k in range(KK):
                    nc.tensor.matmul(
                        nd[:], idt, m[:, k * KB : (k + 1) * KB],
                        start=(k == 0), stop=(k == KK - 1),
                    )

                # epilogue: out_c = num/den + x (interleaved per channel so
                # the tail drains while later channels stream)
                rden = op.tile([128, BW], F32, name="rden")
                nc.vector.reciprocal(rden[:], nd[:, BW : 2 * BW])
                o1 = op.tile([128, BW], F32, name="o1")
                nc.vector.scalar_tensor_tensor(
                    o1[:], nd[:, 0:BW], 1.0, rden[:], ALU.bypass, ALU.mult
                )
                o2 = op.tile([128, BW], F32, name="o2")
                nc.gpsimd.tensor_tensor(
                    o2[:], o1[:], _ap(xf, c * W, [[C * W, BL], [1, W]]), ALU.add
                )
                nc.gpsimd.dma_start(
                    out=bass.AP(yl, c * HWr, [[W, H], [C * HWr, BL], [1, W]]),
                    in_=o2[:],
                )

    _split_wide_waits(nc)
    return nc


_NC_CACHE = None


def _get_nc():
    global _NC_CACHE
    if _NC_CACHE is None:
        _NC_CACHE = _build()
    return _NC_CACHE


def _make_in_maps(x, defocus_map, unet_out, alpha):
    x = np.ascontiguousarray(x, dtype=np.float32)
    defocus_map = np.ascontiguousarray(defocus_map, dtype=np.float32)
    unet_out = np.ascontiguousarray(unet_out, dtype=np.float32)
    alpha_b = np.full((128, 1), np.float32(np.asarray(alpha).reshape(-1)[0]))
    in_maps = []
    for core in range(N_CORES):
        s = slice(core * BL, (core + 1) * BL)
        in_maps.append(
            {
                "x": x[s],
                "defocus": defocus_map[s],
                "unet": unet_out[s],
                "alpha": alpha_b,
            }
        )
    return in_maps


def run(x, defocus_map, unet_out, alpha, **spmd_kwargs):
    """Run the kernel; returns (output, BassKernelResults)."""
    nc = _get_nc()
    in_maps = _make_in_maps(x, defocus_map, unet_out, alpha)
    res = run_bass_kernel_spmd(nc, in_maps, list(range(N_CORES)), **spmd_kwargs)
    out = np.concatenate([res.results[i]["y"] for i in range(N_CORES)], axis=0)
    return out.astype(np.float32), res


def kernel(x, defocus_map, unet_out, alpha):
    return run(x, defocus_map, unet_out, alpha)[0]


# revision 12
# speedup vs baseline: 2.0042x; 1.1459x over previous
"""DefocusLKPN Trainium2 kernel.

Computes, per batch element (reference semantics):
    r      = clip(alpha * defocus + tanh(unet[:,100]), 0, 3)
    disk_k = sigmoid(5*(r - dist_k))            (25 taps, 6 distinct dists)
    w_ck   = exp(l_ck) * disk_k                 (l = unet[:, :100] logits)
    out_c  = sum_k w_ck * patch_ck / sum_k w_ck + x_c

The softmax normalizer of the reference cancels exactly in num/den (the EPS
clamp is never active since the center tap's disk mask is >= 0.5).

Sharding: pure data parallel, batch 16 -> 2 per core across 8 cores.

Per-core layout: partition dim = H (128); free dim packs (b, w) = 256 for
pixel planes and (k, b, w) for the 25-tap weight planes.  The 5x5 unfold is
realized as 5 row-shifted, column-padded copies of x in SBUF (vertical halo)
plus free-dim offsets (horizontal halo).  The row shift itself runs on the
tensor engine (shift-matrix matmuls into PSUM + PSUM->SBUF casts): SBUF->SBUF
shift DMAs measured 14-31 us each on HW and stalled the sync HWDGE FIFO, so
no DMA touches the halo build.  The k-reduction runs on the tensor engine as
identity-matmul accumulation into PSUM (fp16 operands, f32 accumulation).
The disk masks are broadcast once into a 25-tap plane (s25) so the tap-weight
multiplies collapse into a handful of large strided DVE ops; fp16 (not bf16):
the 10-bit mantissa keeps the weighted-average error ~3e-4 of scale.
Logit HBM loads round-robin the three DMA-capable queues (sync/scalar HWDGE,
gpsimd SWDGE) so the 12.5 MiB/core stream saturates HBM.
"""

import sys

sys.path.insert(0, "/opt/trn_rl_repo")

import numpy as np

import concourse.bass as bass
import concourse.mybir as mybir
from concourse.tile import TileContext
from concourse.bass_utils import run_bass_kernel_spmd

F32 = mybir.dt.float32
FP16 = mybir.dt.float16
AF = mybir.ActivationFunctionType
ALU = mybir.AluOpType

MM_DT = FP16

N_CORES = 8
B, C, H, W = 16, 4, 128, 128
BL = B // N_CORES            # 2 batch elements per core
KK = 25
BW = BL * W                  # 256: (b, w) free block
KB = 2 * BW                  # 512: [m | w] block per tap
WP = W + 4                   # 132: padded width per (b, c) block
SDY = BL * C * WP            # 1056: one dy-plane of padded x
DYS = (-2, -1, 0, 1, 2)
KGROUPS = ((0, 9), (9, 8), (17, 8))   # logit planes per DMA queue

# distinct tap distances; k = (dy+2)*5 + (dx+2)
DISTS = [0.0, 1.0, np.sqrt(2.0), 2.0, np.sqrt(5.0), np.sqrt(8.0)]
# (dist_index, base_k, [(step, count), ...]): {base + i*s1 + j*s2} lists the
# taps sharing that dist.
GROUPS = [
    (0, 12, []),                    # dist 0:      {12}
    (1, 7, [(6, 2), (4, 2)]),       # dist 1:      {7, 11, 13, 17}
    (2, 6, [(10, 2), (2, 2)]),      # dist sqrt2:  {6, 8, 16, 18}
    (3, 2, [(12, 2), (8, 2)]),      # dist 2:      {2, 10, 14, 22}
    (4, 5, [(10, 2), (4, 2)]),      # dist sqrt5:  {5, 9, 15, 19}
    (4, 1, [(20, 2), (2, 2)]),      # dist sqrt5:  {1, 3, 21, 23}
    (5, 0, [(20, 2), (4, 2)]),      # dist sqrt8:  {0, 4, 20, 24}
]


def _split_wide_waits(nc, max_waits=1):
    """The walrus build here accepts at most one semaphore wait per
    instruction; move extra waits onto preceding Drains on the same engine."""
    n = 0
    for func in nc.m.functions:
        for bb in func.blocks:
            out = []
            changed = False
            for ins in bb.instructions:
                si = ins.sync_info
                if si is not None and si.on_wait and len(si.on_wait) > max_waits:
                    waits = list(si.on_wait)
                    keep, rest = waits[:max_waits], waits[max_waits:]
                    for i in range(0, len(rest), max_waits):
                        n += 1
                        out.append(
                            mybir.InstDrain(
                                name=f"splitwait-{n}",
                                opcode="Drain",
                                engine=ins.engine,
                                sync_info=mybir.SyncInfo(
                                    on_wait=list(rest[i : i + max_waits]),
                                    on_update=[],
                                ),
                            )
                        )
                    si.on_wait = keep
                    changed = True
                out.append(ins)
            if changed:
                bb.instructions = out
    return n


def _ap(t, extra_off, dims):
    """AP over tile `t` keeping its partition dim, with free dims
    [[step, count], ...] in elements and an extra element offset."""
    return bass.AP(t.tensor, t.offset + extra_off, [list(t.ap[0])] + [list(d) for d in dims])


def _shift_mats():
    """[128, 5*128] fp16: column block dyi holds S with S[q, p] = 1 iff
    q == p + dy, so out[p] = (S.T @ x)[p] = x[p + dy] (zero out of range)."""
    sh = np.zeros((128, 5 * 128), dtype=np.float16)
    for dyi, dy in enumerate(DYS):
        for p in range(128):
            q = p + dy
            if 0 <= q < 128:
                sh[q, dyi * 128 + p] = 1.0
    return sh


def _build():
    nc = bass.Bass("TRN2", num_devices=N_CORES)

    xl = nc.dram_tensor("x", [BL, C, H, W], F32, kind="ExternalInput")
    dfl = nc.dram_tensor("defocus", [BL, 1, H, W], F32, kind="ExternalInput")
    ul = nc.dram_tensor("unet", [BL, 4 * KK + 1, H, W], F32, kind="ExternalInput")
    al = nc.dram_tensor("alpha", [128, 1], F32, kind="ExternalInput")
    yl = nc.dram_tensor("y", [BL, C, H, W], F32, kind="ExternalOutput")

    shifts_dram = nc.inline_tensor(_shift_mats(), name="shifts")

    UCH = ul.shape[1]          # 101
    HWr = H * W                # plane stride in DRAM
    dma_engines = [None, None, None]  # filled inside (sync/scalar/gpsimd)

    with TileContext(nc) as tc:
        with (
            tc.tile_pool(name="fix", bufs=1) as fix,
            tc.tile_pool(name="lp", bufs=2) as lp,
            tc.tile_pool(name="ep", bufs=2) as ep,
            tc.tile_pool(name="mp", bufs=2) as mp,
            tc.tile_pool(name="op", bufs=2) as op,
            tc.tile_pool(name="ps", bufs=1, space="PSUM") as ps,
            tc.tile_pool(name="psx", bufs=2, space="PSUM") as psx,
        ):
            dma_engines = [nc.sync, nc.scalar, nc.gpsimd]
            # ---- constants / prologue ------------------------------------
            # radius-chain inputs first so their bytes hit the wire earliest
            asb = fix.tile([128, 1], F32)
            nc.sync.dma_start(out=asb[:], in_=al[:])
            df = fix.tile([128, BW], F32)
            nc.sync.dma_start(
                out=df[:],
                in_=bass.AP(dfl, 0, [[W, H], [HWr, BL], [1, W]]),
            )
            u100 = fix.tile([128, BW], F32)
            nc.sync.dma_start(
                out=u100[:],
                in_=bass.AP(ul, 100 * HWr, [[W, H], [UCH * HWr, BL], [1, W]]),
            )
            xf = fix.tile([128, BL * C * W], F32)
            nc.sync.dma_start(
                out=xf[:],
                in_=bass.AP(xl, 0, [[W, H], [HWr, BL * C], [1, W]]),
            )
            shf = fix.tile([128, 5 * 128], MM_DT)
            nc.sync.dma_start(out=shf[:], in_=shifts_dram[:])
            idt = shf[:, 2 * 128 : 3 * 128]          # dy=0 block == identity

            # x cast first so it leads the DVE queue (depends only on xf)
            xb = fix.tile([128, BL * C * W], MM_DT)
            nc.vector.tensor_copy(xb[:], xf[:])

            # radius = clip(alpha*defocus + tanh(u100), 0, 3)
            dtan = fix.tile([128, BW], F32)
            nc.scalar.activation(dtan[:], u100[:], AF.Tanh)
            r0 = fix.tile([128, BW], F32)
            nc.vector.scalar_tensor_tensor(r0[:], df[:], asb[:, :1], dtan[:], ALU.mult, ALU.add)
            rr = fix.tile([128, BW], F32)
            nc.vector.tensor_scalar(rr[:], r0[:], 0.0, 3.0, ALU.max, ALU.min)

            # s6[d] = sigmoid(5*r - 5*dist_d)   (6 planes, shared by all c)
            bt = fix.tile([128, 6], F32)
            for d in range(6):
                nc.gpsimd.memset(bt[:, d : d + 1], float(-5.0 * DISTS[d]))
            s6 = fix.tile([128, 6 * BW], MM_DT)
            for d in range(6):
                nc.scalar.activation(
                    s6[:, d * BW : (d + 1) * BW], rr[:], AF.Sigmoid,
                    bias=bt[:, d : d + 1], scale=5.0,
                )
            # broadcast the 6 dist planes into per-tap order (cheap on DVE;
            # gpsimd's Q7 runs ~4 ns/elem and serialized 21 us here)
            s25 = fix.tile([128, KK * BW], MM_DT)
            for d, base, steps in GROUPS:
                sdims = [[s * BW, cnt] for s, cnt in steps] + [[1, BW]]
                bdims = [[0, cnt] for _, cnt in steps] + [[1, BW]]
                nc.vector.tensor_copy(
                    _ap(s25, base * BW, sdims), _ap(s6, d * BW, bdims)
                )

            # 5 row-shifted padded copies of x: dy=0 is a plain strided copy
            # of xb; dy!=0 run on the PE (shift-matrix matmul into PSUM, then
            # cast into the padded tile).  Even dx taps read 4B-aligned
            # slices (DVE 2x); odd dx taps read the same tile at odd offsets
            # in 1x mode (cheaper than keeping a second odd-aligned copy).
            xs_all = fix.tile([128, 5 * SDY], MM_DT)
            nc.gpsimd.memset(_ap(xs_all, 0, [[SDY, 5], [WP, BL * C], [1, 2]]), 0.0)
            nc.gpsimd.memset(
                _ap(xs_all, 2 + W, [[SDY, 5], [WP, BL * C], [1, 2]]), 0.0
            )
            nc.vector.tensor_copy(
                _ap(xs_all, 2 * SDY + 2, [[WP, BL * C], [1, W]]),
                _ap(xb, 0, [[W, BL * C], [1, W]]),
            )
            for dyi in (0, 1, 3, 4):
                xps = psx.tile([128, BL * C * W], F32, name="xps")
                for h in range(2):
                    nc.tensor.matmul(
                        xps[:, h * 512 : (h + 1) * 512],
                        shf[:, dyi * 128 : (dyi + 1) * 128],
                        xb[:, h * 512 : (h + 1) * 512],
                        start=True, stop=True,
                    )
                nc.vector.tensor_copy(
                    _ap(xs_all, dyi * SDY + 2, [[WP, BL * C], [1, W]]),
                    _ap(xps, 0, [[W, BL * C], [1, W]]),
                )

            # ---- per-channel main loop -----------------------------------
            # numden[c] accumulates [num | den]; each matmul's rhs is one
            # tap's [m_k (256) | w_k (256)] block (N=512 = one PSUM bank).
            numdens = []
            for c in range(C):
                nd = ps.tile([128, KB], F32, name=f"numden{c}")
                numdens.append(nd)

                # per-kgroup l tiles: c+2's load of group g waits only on
                # c's exp of group g (not the whole channel) -> no DMA stall
                ls = []
                for gi, (k0, nk) in enumerate(KGROUPS):
                    lg = lp.tile([128, nk * BW], F32, name=f"l{gi}")
                    ls.append(lg)
                    for b in range(BL):
                        dma_engines[gi].dma_start(
                            out=_ap(lg, b * W, [[BW, nk], [1, W]]),
                            in_=bass.AP(
                                ul, (c * KK + k0 + b * UCH) * HWr,
                                [[W, H], [HWr, nk], [1, W]],
                            ),
                        )
                lexp = ep.tile([128, KK * BW], MM_DT, name="lexp")
                for gi, (k0, nk) in enumerate(KGROUPS):
                    nc.scalar.activation(
                        lexp[:, k0 * BW : (k0 + nk) * BW], ls[gi][:], AF.Exp,
                    )

                m = mp.tile([128, KK * KB], MM_DT, name="mall")
                # w_k = s25_k * lexp_k into the w half of each tap block
                for k0, nk in KGROUPS:
                    nc.vector.tensor_tensor(
                        _ap(m, k0 * KB + BW, [[KB, nk], [1, BW]]),
                        _ap(s25, k0 * BW, [[BW, nk], [1, BW]]),
                        _ap(lexp, k0 * BW, [[BW, nk], [1, BW]]),
                        ALU.mult,
                    )
                # m_k = w_k * x[h+dy, w+dx], per dy (both b per op) so the
                # 5 taps' matmuls can stream right behind each dy's product
                # (even dx 4B-aligned = DVE 2x; odd dx offsets run 1x)
                for dy in range(5):
                    nc.vector.tensor_tensor(
                        _ap(m, 5 * KB * dy, [[2 * KB, 3], [W, BL], [1, W]]),
                        _ap(m, 5 * KB * dy + BW, [[2 * KB, 3], [W, BL], [1, W]]),
                        _ap(xs_all, dy * SDY + c * WP, [[2, 3], [C * WP, BL], [1, W]]),
                        ALU.mult,
                    )
                    nc.vector.tensor_tensor(
                        _ap(m, 5 * KB * dy + KB, [[2 * KB, 2], [W, BL], [1, W]]),
                        _ap(m, 5 * KB * dy + KB + BW, [[2 * KB, 2], [W, BL], [1, W]]),
                        _ap(xs_all, dy * SDY + c * WP + 1, [[2, 2], [C * WP, BL], [1, W]]),
                        ALU.mult,
                    )
                    for j in range(5):
                        k = 5 * dy + j
                        nc.tensor.matmul(
                            nd[:], idt, m[:, k * KB : (k + 1) * KB],
                            start=(k == 0), stop=(k == KK - 1),
                        )

            # ---- epilogue: out_c = num/den + x ---------------------------
            # 1/den = exp(-ln den) on ACT (den > 0 always; bass bans the
            # direct Reciprocal table).  Deferred past the last tap exp and
            # batched ln->exp so the act-table swaps at most twice; this
            # keeps the slow DVE reciprocal off the drain path.  o2 + store
            # drain on gpsimd.
            ldens, rdens = [], []
            for c in range(C):
                lden = op.tile([128, BW], F32, name=f"lden{c}", bufs=1)
                nc.scalar.activation(lden[:], numdens[c][:, BW : 2 * BW], AF.Ln)
                ldens.append(lden)
            for c in range(C):
                rden = op.tile([128, BW], F32, name=f"rden{c}", bufs=1)
                nc.scalar.activation(rden[:], ldens[c][:], AF.Exp, scale=-1.0)
                rdens.append(rden)
            for c in range(C):
                rden = rdens[c]
                o1 = op.tile([128, BW], F32, name="o1")
                nc.vector.scalar_tensor_tensor(
                    o1[:], numdens[c][:, 0:BW], 1.0, rden[:], ALU.bypass, ALU.mult
                )
                o2 = op.tile([128, BW], F32, name="o2")
                nc.gpsimd.tensor_tensor(
                    o2[:], o1[:], _ap(xf, c * W, [[C * W, BL], [1, W]]), ALU.add
                )
                nc.gpsimd.dma_start(
                    out=bass.AP(yl, c * HWr, [[W, H], [C * HWr, BL], [1, W]]),
                    in_=o2[:],
                )

    _split_wide_waits(nc)
    return nc


_NC_CACHE = None


def _get_nc():
    global _NC_CACHE
    if _NC_CACHE is None:
        _NC_CACHE = _build()
    return _NC_CACHE


def _make_in_maps(x, defocus_map, unet_out, alpha):
    x = np.ascontiguousarray(x, dtype=np.float32)
    defocus_map = np.ascontiguousarray(defocus_map, dtype=np.float32)
    unet_out = np.ascontiguousarray(unet_out, dtype=np.float32)
    alpha_b = np.full((128, 1), np.float32(np.asarray(alpha).reshape(-1)[0]))
    in_maps = []
    for core in range(N_CORES):
        s = slice(core * BL, (core + 1) * BL)
        in_maps.append(
            {
                "x": x[s],
                "defocus": defocus_map[s],
                "unet": unet_out[s],
                "alpha": alpha_b,
            }
        )
    return in_maps


def run(x, defocus_map, unet_out, alpha, **spmd_kwargs):
    """Run the kernel; returns (output, BassKernelResults)."""
    nc = _get_nc()
    in_maps = _make_in_maps(x, defocus_map, unet_out, alpha)
    res = run_bass_kernel_spmd(nc, in_maps, list(range(N_CORES)), **spmd_kwargs)
    out = np.concatenate([res.results[i]["y"] for i in range(N_CORES)], axis=0)
    return out.astype(np.float32), res


def kernel(x, defocus_map, unet_out, alpha):
    return run(x, defocus_map, unet_out, alpha)[0]
